# revision 9
# baseline (speedup 1.0000x reference)
"""Differentiable ECE (soft histogram binning) on 8 trn2 NeuronCores.

Math: reference computes, for 10 bin centers c_b = 0.05 + 0.1*b,
    w_b(p) = exp(-(p-c_b)^2 / 0.02)           (1/0.02 = 50)
    S_b = sum_n w_b;  C_b = sum_n w_b p_n;  A_b = sum_n w_b l_n
    ECE = sum_b (S_b/(S_b+eps)) * |C_b - A_b| / (S_b+eps)

Kernel strategy (per core, data-parallel over the flattened element axis):
  * S_b: the scalar engine evaluates Derivative_Erf(sqrt(50)*(p-c_b)) =
    (2/sqrt(pi)) * exp(-50 (p-c_b)^2) in ONE activation pass per bin with
    fused per-partition accumulation (accum_out).  10 ACT passes/chunk.
  * C_b, A_b: identity w_b = w_0 * r^b * Q_b with r = exp(10p) (host-
    precomputed, bf16) and scalar Q_b = exp(-b(b+1)/2).  Two bf16 multiply
    chains on the vector engine (up_b = w0*p*r^b, ul_b = w0*l*r^b), one
    2x-mode tensor_tensor per bin each.  20 DVE passes/chunk.
  * Reductions for up/ul: tensor engine matmuls with one-hot stationary
    matrices accumulate every chain tile into a [20, 512] PSUM region
    (start=False accumulation across all chunks).
  * Host: finishes the ~20*512 + 80*128 partial sums in float64, applies
    the sqrt(pi)/2 and Q_b constants, and evaluates the ECE formula.
"""

import sys

sys.path.insert(0, "/opt/trn_rl_repo")

import math
from contextlib import ExitStack

import ml_dtypes
import numpy as np

import concourse.bass as bass
import concourse.tile as tile
from concourse import bacc, mybir
from concourse.bass_utils import run_bass_kernel_spmd

N_CORES = 8
P_DIM = 128
ROWS, COLS = 2048, 8192
F_TOT = ROWS * COLS // N_CORES // P_DIM  # 16384 free elems per partition per core
F = 4096                                 # chunk free size
NCH = F_TOT // F
NB = 10                                  # bins
NQ = 2 * NB                              # 20 matmul-reduced quantities (up, ul)
J = 512                                  # matmul moving free dim
EPS = 1e-8
SQ50 = math.sqrt(50.0)

_cache = {}


def _build():
    nc = bacc.Bacc("TRN2", target_bir_lowering=False, debug=False)
    f32, bf16 = mybir.dt.float32, mybir.dt.bfloat16
    Act = mybir.ActivationFunctionType

    # Register const APs for the activation biases -sqrt(50)*c_b (activation()
    # requires non-Copy bias as a const AP, same mechanism as Bass.__init__).
    centers = [0.05 + 0.1 * b for b in range(NB)]
    biases = [np.float32(-SQ50 * c) for c in centers]
    for i, v in enumerate(biases):
        t = nc.alloc_sbuf_tensor(f"const-bias-{i}", [128, 1], f32)
        nc.gpsimd.memset(t.ap(), float(v))
        nc.const_aps.aps[(f32, float(v))] = t.ap()
    nc.all_engine_barrier()

    p32 = nc.dram_tensor("p32", [P_DIM, F_TOT], f32, kind="ExternalInput").ap()
    pb = nc.dram_tensor("pb", [P_DIM, F_TOT], bf16, kind="ExternalInput").ap()
    lb = nc.dram_tensor("lb", [P_DIM, F_TOT], bf16, kind="ExternalInput").ap()
    rb = nc.dram_tensor("rb", [P_DIM, F_TOT], bf16, kind="ExternalInput").ap()
    emat = nc.dram_tensor("emat", [P_DIM, NQ * NQ], bf16, kind="ExternalInput").ap()
    acc = nc.dram_tensor("acc", [NQ, J], f32, kind="ExternalOutput").ap()
    accs = nc.dram_tensor("accs", [P_DIM, NB * NCH], f32, kind="ExternalOutput").ap()

    n_mm_total = NCH * NQ * (F // J)

    with tile.TileContext(nc) as tc, ExitStack() as ctx:
        pool_c = ctx.enter_context(tc.tile_pool(name="const", bufs=1))
        pool_p = ctx.enter_context(tc.tile_pool(name="p", bufs=2))
        pool_b = ctx.enter_context(tc.tile_pool(name="b", bufs=2))
        pool_w = ctx.enter_context(tc.tile_pool(name="w", bufs=3))
        pool_ps = ctx.enter_context(tc.tile_pool(name="ps", bufs=1, space="PSUM"))

        em = pool_c.tile([P_DIM, NQ * NQ], bf16)
        nc.gpsimd.dma_start(em[:], emat[:])
        ps = pool_ps.tile([NQ, J], f32)
        accs_t = pool_c.tile([P_DIM, NB * NCH], f32)
        junk = pool_c.tile([P_DIM, F], bf16)

        mm_count = [0]

        def reduce_into(row, t):
            for j0 in range(0, F, J):
                i = mm_count[0]
                nc.tensor.matmul(
                    ps[:, :],
                    em[:, row * NQ : (row + 1) * NQ],
                    t[:, j0 : j0 + J],
                    start=(i == 0),
                    stop=(i == n_mm_total - 1),
                )
                mm_count[0] += 1

        for ci in range(NCH):
            sl = slice(ci * F, (ci + 1) * F)
            pf = pool_p.tile([P_DIM, F], f32, tag="pf")
            nc.gpsimd.dma_start(pf[:], p32[:, sl])
            pbt = pool_b.tile([P_DIM, F], bf16, tag="pb")
            nc.gpsimd.dma_start(pbt[:], pb[:, sl])
            lbt = pool_b.tile([P_DIM, F], bf16, tag="lb")
            nc.gpsimd.dma_start(lbt[:], lb[:, sl])
            rbt = pool_b.tile([P_DIM, F], bf16, tag="rb")
            nc.gpsimd.dma_start(rbt[:], rb[:, sl])

            # u0 = (2/sqrt(pi)) exp(-50 (p-0.05)^2), S'_0 accumulated
            u0 = pool_w.tile([P_DIM, F], bf16, tag="u0")
            nc.scalar.activation(
                u0[:], pf[:], Act.Derivative_Erf,
                bias=float(biases[0]), scale=SQ50,
                accum_out=accs_t[:, ci * NB : ci * NB + 1],
            )
            # S'_b for b=1..9: accumulate-only Derivative_Erf passes
            for b in range(1, NB):
                nc.scalar.activation(
                    junk[:], pf[:], Act.Derivative_Erf,
                    bias=float(biases[b]), scale=SQ50,
                    accum_out=accs_t[:, ci * NB + b : ci * NB + b + 1],
                )

            up = pool_w.tile([P_DIM, F], bf16, tag="up")
            nc.vector.tensor_mul(up[:], u0[:], pbt[:])
            ul = pool_w.tile([P_DIM, F], bf16, tag="ul")
            nc.vector.tensor_mul(ul[:], u0[:], lbt[:])
            reduce_into(0, up)
            reduce_into(NB, ul)

            for b in range(1, NB):
                up2 = pool_w.tile([P_DIM, F], bf16, tag="up")
                nc.vector.tensor_mul(up2[:], up[:], rbt[:])
                up = up2
                ul2 = pool_w.tile([P_DIM, F], bf16, tag="ul")
                nc.vector.tensor_mul(ul2[:], ul[:], rbt[:])
                ul = ul2
                reduce_into(b, up)
                reduce_into(NB + b, ul)

        outsb = pool_c.tile([NQ, J], f32)
        nc.vector.tensor_copy(outsb[:], ps[:])
        nc.gpsimd.dma_start(acc[:], outsb[:])
        nc.gpsimd.dma_start(accs[:], accs_t[:])

    nc.finalize()
    return nc


def _get_nc():
    if "nc" not in _cache:
        _cache["nc"] = _build()
    return _cache["nc"]


def kernel(probs, labels):
    nc = _get_nc()

    p = np.ascontiguousarray(np.asarray(probs, dtype=np.float32)).reshape(
        N_CORES, P_DIM, F_TOT
    )
    pbf = p.astype(ml_dtypes.bfloat16)
    rbf = np.exp(10.0 * p).astype(ml_dtypes.bfloat16)
    lbf = (
        np.ascontiguousarray(np.asarray(labels))
        .reshape(N_CORES, P_DIM, F_TOT)
        .astype(ml_dtypes.bfloat16)
    )
    em = np.zeros((NQ, NQ), dtype=ml_dtypes.bfloat16)
    np.fill_diagonal(em, 1.0)
    em = np.tile(em.reshape(1, NQ * NQ), (P_DIM, 1))

    in_maps = [
        {"p32": p[i], "pb": pbf[i], "lb": lbf[i], "rb": rbf[i], "emat": em}
        for i in range(N_CORES)
    ]
    res = run_bass_kernel_spmd(nc, in_maps, list(range(N_CORES)))

    rows = np.zeros(NQ, dtype=np.float64)
    s_rows = np.zeros(NB, dtype=np.float64)
    for i in range(N_CORES):
        rows += res.results[i]["acc"].astype(np.float64).sum(axis=1)
        a = res.results[i]["accs"].astype(np.float64).reshape(P_DIM, NCH, NB)
        s_rows += a.sum(axis=(0, 1))

    b = np.arange(NB, dtype=np.float64)
    Q = np.exp(-0.5 * (b * b + b))
    HALF_SQRT_PI = math.sqrt(math.pi) / 2.0
    S = s_rows * HALF_SQRT_PI
    # up/ul chains start from u0 = (2/sqrt(pi)) w0, so C/A carry that factor too
    C = rows[0:NB] * Q * HALF_SQRT_PI
    A = rows[NB : 2 * NB] * Q * HALF_SQRT_PI

    denom = S + EPS
    ece = ((S / denom) * np.abs(C - A) / denom).sum()
    return np.float32(ece)
